# revision 8
# baseline (speedup 1.0000x reference)
"""Trainium2 Bass kernel for a GAT block.

Math (after algebraic simplification of the reference):
  h[b,f,n,k] = x[b,:,f,n] @ W[:,k] + bW[k]
  s2[b,f,n]  = h[b,f,n,:] @ a2          (the s1/ab terms cancel in softmax)
  d[b,f,n]   = softmax_n(s2)[n] * mask[n,n]
  out[b,k,f,n] = d[b,f,n] * h[b,f,n,k]

Sharding: data-parallel over batch, 4 batches per core on 8 cores.

Device layout per core (B'=4, C=3, F=2048, N=25, H=64):
  - frames processed in q-units of 512; each psum tile [128,400] holds
    rows 0:64 = H x 16 frames from the q-unit's first 256 frames,
    rows 64:128 = H x 16 frames from the last 256 (so all 128 partitions
    work and the per-half flatten of d stays contiguous).
  - h via PE matmul (lhsT=W [3,64]); d replicated across partitions via a
    second PE matmul (lhsT=sel2 [2,128], rhs = flattened d rows [2,400]).
  - softmax computed with frames on partitions ([128, 4, 25] per q-unit),
    then flattened to rows with one SBUF->SBUF DMA per 128-frame sweep.
"""

import sys

if "/opt/trn_rl_repo" not in sys.path:
    sys.path.insert(0, "/opt/trn_rl_repo")

import numpy as np

B, C, F, N, H = 32, 3, 2048, 25, 64
NCORES = 8
BPC = B // NCORES  # batches per core
QF = 512           # frames per q-unit
NQ = F // QF       # q-units per batch
HFN = 128 * N      # 3200, elements per 128-frame sweep

_NC_CACHE = {}


def _build_nc():
    import concourse.bass as bass
    import concourse.bacc as bacc
    import concourse.tile as tile
    from concourse import mybir

    f32 = mybir.dt.float32
    MULT = mybir.AluOpType.mult
    ADD = mybir.AluOpType.add
    AX = mybir.AxisListType.X
    EXP = mybir.ActivationFunctionType.Exp
    IDENT = mybir.ActivationFunctionType.Identity

    nc = bacc.Bacc()
    x_d = nc.declare_dram_parameter("x", [BPC, C, F, N], f32, isOutput=False)
    w_d = nc.declare_dram_parameter("w", [C, H], f32, isOutput=False)
    bw_d = nc.declare_dram_parameter("bw_pp", [128, 1], f32, isOutput=False)
    v_d = nc.declare_dram_parameter("v_pp", [128, C], f32, isOutput=False)
    c0_d = nc.declare_dram_parameter("c0_pp", [128, 1], f32, isOutput=False)
    md_d = nc.declare_dram_parameter("md4", [128, 4, N], f32, isOutput=False)
    sel_d = nc.declare_dram_parameter("sel2", [2, 128], f32, isOutput=False)
    out_d = nc.declare_dram_parameter("out", [BPC, H, F, N], f32, isOutput=True)

    with tile.TileContext(nc) as tc:
        with (
            tc.tile_pool(name="singles", bufs=1) as singles,
            tc.tile_pool(name="xfp", bufs=2) as xfp_pool,
            tc.tile_pool(name="sm", bufs=2) as sm_pool,
            tc.tile_pool(name="dflat", bufs=2) as dflat_pool,
            tc.tile_pool(name="xc", bufs=2) as xc_pool,
            tc.tile_pool(name="osb", bufs=2) as osb_pool,
            tc.tile_pool(name="hsb", bufs=3) as hsb_pool,
            tc.tile_pool(name="ps", bufs=2, space="PSUM") as ps_pool,
        ):
            w_sb = singles.tile([C, H], f32)
            nc.sync.dma_start(out=w_sb[:], in_=w_d[:, :])
            sel_sb = singles.tile([2, 128], f32)
            nc.sync.dma_start(out=sel_sb[:], in_=sel_d[:, :])
            bw_sb = singles.tile([128, 1], f32)
            nc.sync.dma_start(out=bw_sb[:], in_=bw_d[:, :])
            v_sb = singles.tile([128, C], f32)
            nc.sync.dma_start(out=v_sb[:], in_=v_d[:, :])
            c0_sb = singles.tile([128, 1], f32)
            nc.sync.dma_start(out=c0_sb[:], in_=c0_d[:, :])
            md_sb = singles.tile([128, 4, N], f32)
            nc.sync.dma_start(out=md_sb[:], in_=md_d[:, :, :])

            for b in range(BPC):
                for q in range(NQ):
                    f0 = q * QF
                    # ---- softmax phase: frames on partitions ----
                    xfp = xfp_pool.tile([128, 4, C, N], f32)
                    for fb in range(4):
                        src = x_d[b, :, f0 + 128 * fb : f0 + 128 * (fb + 1), :]
                        nc.sync.dma_start(
                            out=xfp[:, fb, :, :], in_=src.transpose([1, 0, 2])
                        )
                    t = sm_pool.tile([128, 4, N], f32, tag="t")
                    nc.vector.tensor_scalar(
                        out=t[:],
                        in0=xfp[:, :, 2, :],
                        scalar1=v_sb[:, 2:3],
                        scalar2=c0_sb[:, :],
                        op0=MULT,
                        op1=ADD,
                    )
                    for c in (1, 0):
                        nc.vector.scalar_tensor_tensor(
                            out=t[:],
                            in0=xfp[:, :, c, :],
                            scalar=v_sb[:, c : c + 1],
                            in1=t[:],
                            op0=MULT,
                            op1=ADD,
                        )
                    e = sm_pool.tile([128, 4, N], f32, tag="e")
                    nc.scalar.activation(out=e[:], in_=t[:], func=EXP)
                    z = sm_pool.tile([128, 4], f32, tag="z")
                    nc.vector.reduce_sum(out=z[:], in_=e[:], axis=AX)
                    r = sm_pool.tile([128, 4], f32, tag="r")
                    nc.vector.reciprocal(out=r[:], in_=z[:])
                    dq = sm_pool.tile([128, 4, N], f32, tag="dq")
                    nc.vector.tensor_tensor(
                        out=dq[:], in0=e[:], in1=md_sb[:], op=MULT
                    )
                    for fb in range(4):
                        nc.vector.tensor_scalar_mul(
                            out=dq[:, fb, :],
                            in0=dq[:, fb, :],
                            scalar1=r[:, fb : fb + 1],
                        )
                    # ---- flatten d to [2, 6400] rows ----
                    dflat = dflat_pool.tile([2, 2 * HFN], f32)
                    for fb in range(4):
                        i, s = fb // 2, fb % 2
                        dst = dflat[
                            i : i + 1, s * HFN : (s + 1) * HFN
                        ].rearrange("a (p n) -> a p n", n=N)
                        nc.gpsimd.dma_start(out=dst, in_=dq[:, fb, :])
                    # ---- compute phase: 2 half-units of 8 tiles ----
                    for hh in range(2):
                        xc = xc_pool.tile([C, 2 * HFN], f32)
                        fa = f0 + 128 * hh
                        fb_ = f0 + 256 + 128 * hh
                        nc.scalar.dma_start(
                            out=xc[:, 0:HFN], in_=x_d[b, :, fa : fa + 128, :]
                        )
                        nc.scalar.dma_start(
                            out=xc[:, HFN:], in_=x_d[b, :, fb_ : fb_ + 128, :]
                        )
                        osb = osb_pool.tile([128, 8, 400], f32)
                        for tt in range(8):
                            tp = 8 * hh + tt
                            ph = ps_pool.tile([128, 400], f32, tag="ph")
                            nc.tensor.matmul(
                                ph[0:64, :],
                                w_sb[:],
                                xc[:, 400 * tt : 400 * (tt + 1)],
                                start=True,
                                stop=True,
                            )
                            nc.tensor.matmul(
                                ph[64:128, :],
                                w_sb[:],
                                xc[:, HFN + 400 * tt : HFN + 400 * (tt + 1)],
                                start=True,
                                stop=True,
                            )
                            pd = ps_pool.tile([128, 400], f32, tag="pd")
                            nc.tensor.matmul(
                                pd[:],
                                sel_sb[:],
                                dflat[:, 400 * tp : 400 * (tp + 1)],
                                start=True,
                                stop=True,
                            )
                            hsb = hsb_pool.tile([128, 400], f32)
                            nc.scalar.activation(
                                out=hsb[:],
                                in_=ph[:],
                                func=IDENT,
                                bias=bw_sb[:, :],
                            )
                            nc.vector.tensor_tensor(
                                out=osb[:, tt, :],
                                in0=hsb[:],
                                in1=pd[:],
                                op=MULT,
                            )
                        nc.sync.dma_start(
                            out=out_d[b, :, fa : fa + 128, :],
                            in_=osb[0:64, :, :],
                        )
                        nc.sync.dma_start(
                            out=out_d[b, :, fb_ : fb_ + 128, :],
                            in_=osb[64:128, :, :],
                        )
    nc.compile()
    return nc


def _get_nc():
    if "nc" not in _NC_CACHE:
        _NC_CACHE["nc"] = _build_nc()
    return _NC_CACHE["nc"]


def _make_in_maps(x, mask, W, bW, a1, a2, ab):
    x = np.ascontiguousarray(np.asarray(x, np.float32))
    mask = np.asarray(mask, np.float32)
    W = np.ascontiguousarray(np.asarray(W, np.float32))
    bW = np.asarray(bW, np.float32)
    a2 = np.asarray(a2, np.float32)

    v = (W @ a2).astype(np.float32)                      # [C]
    c0 = np.float32(bW @ a2)
    md = np.ascontiguousarray(np.diag(mask)).astype(np.float32)  # [N]

    bw_pp = np.concatenate([bW, bW]).reshape(128, 1).astype(np.float32)
    v_pp = np.tile(v[None, :], (128, 1)).astype(np.float32)
    c0_pp = np.full((128, 1), c0, np.float32)
    md4 = np.ascontiguousarray(
        np.tile(md[None, None, :], (128, 4, 1)).astype(np.float32)
    )
    sel2 = np.zeros((2, 128), np.float32)
    sel2[0, :64] = 1.0
    sel2[1, 64:] = 1.0

    in_maps = []
    for c in range(NCORES):
        in_maps.append(
            {
                "x": np.ascontiguousarray(x[c * BPC : (c + 1) * BPC]),
                "w": W,
                "bw_pp": bw_pp,
                "v_pp": v_pp,
                "c0_pp": c0_pp,
                "md4": md4,
                "sel2": sel2,
            }
        )
    return in_maps


def run(x, mask, W, bW, a1, a2, ab, **run_kwargs):
    from concourse.bass_utils import run_bass_kernel_spmd

    nc = _get_nc()
    in_maps = _make_in_maps(x, mask, W, bW, a1, a2, ab)
    res = run_bass_kernel_spmd(nc, in_maps, core_ids=list(range(NCORES)), **run_kwargs)
    out = np.concatenate([res.results[i]["out"] for i in range(NCORES)], axis=0)
    return out, res


def kernel(x, mask, W, bW, a1, a2, ab):
    out, _ = run(x, mask, W, bW, a1, a2, ab)
    return out


# revision 9
# speedup vs baseline: 2.2967x; 2.2967x over previous
"""Trainium2 Bass kernel for a GAT block.

Math (after algebraic simplification of the reference):
  h[b,f,n,k] = x[b,:,f,n] @ W[:,k] + bW[k]
  s2[b,f,n]  = h[b,f,n,:] @ a2 = v.x + c0   (s1/ab cancel inside softmax)
  d[b,f,n]   = softmax_n(s2)[n] * mask[n,n]
  out[b,k,f,n] = d[b,f,n] * h[b,f,n,k] = sum_c W[c,k] (x*d)[c,f,n] + bW[k] d[f,n]

Sharding: data-parallel over batch, 4 batches per core on 8 cores.

Device pipeline per (batch, 512-frame q-unit), all shapes [partitions, free]:
  1. x_s  [32, 3, 400]   one DMA, partition = 16-frame group "fsub"
  2. softmax on DVE/ACT -> dd [32, 400] = d in (fsub | f', n) layout
  3. dmult [128, 400] = dd stacked 4x (4 sbuf copies)
  4. x4   [128, 400]: rows 32c+fsub = x[c], rows 96:128 = 1.0 (memset)
  5. x4s = x4 * dmult    (one DVE op: x*d rows + d rows)
  6. 16 matmuls: psum[128,400] = wsel[tp].T @ x4s, where wsel[tp] [128,128]
     selects the tile's two fsubs and applies [W; bW] -> psum = final out
     for 32 frames: rows (jj,k), cols (f',n)
  7. evict psum -> out_sb (DVE/ACT alternating), 2 store DMAs per q-unit
"""

import sys

if "/opt/trn_rl_repo" not in sys.path:
    sys.path.insert(0, "/opt/trn_rl_repo")

import numpy as np

B, C, F, N, H = 32, 3, 2048, 25, 64
NCORES = 8
BPC = B // NCORES   # batches per core
QF = 512            # frames per q-unit
NQ = F // QF        # q-units per batch
FSUB = 16           # frames per fsub row
NS = QF // FSUB     # 32 fsub rows per q-unit
FN = F * N
TW = FSUB * N       # 400, columns per tile

# matmul operand dtype: "f32" (exact) or "f32r" (~2e-4, 4x faster PE)
MM_DTYPE = "f32"

_NC_CACHE = {}


def _build_nc():
    import concourse.bass as bass
    import concourse.bacc as bacc
    import concourse.tile as tile
    from concourse import mybir

    f32 = mybir.dt.float32
    mmdt = f32 if MM_DTYPE == "f32" else mybir.dt.float32r
    MULT = mybir.AluOpType.mult
    ADD = mybir.AluOpType.add
    AX = mybir.AxisListType.X
    EXP = mybir.ActivationFunctionType.Exp

    nc = bacc.Bacc()
    x_d = nc.declare_dram_parameter("x", [BPC, C, F, N], f32, isOutput=False)
    wsel_d = nc.declare_dram_parameter("wsel", [128, NS // 2, 128], mmdt, isOutput=False)
    v_d = nc.declare_dram_parameter("v_pp", [NS, C], f32, isOutput=False)
    c0_d = nc.declare_dram_parameter("c0_pp", [NS, 1], f32, isOutput=False)
    md_d = nc.declare_dram_parameter("mdt", [NS, TW], f32, isOutput=False)
    out_d = nc.declare_dram_parameter("out", [BPC, H, F, N], f32, isOutput=True)

    with tile.TileContext(nc) as tc:
        with (
            tc.tile_pool(name="singles", bufs=1) as singles,
            tc.tile_pool(name="xs", bufs=2) as xs_pool,
            tc.tile_pool(name="sm", bufs=2) as sm_pool,
            tc.tile_pool(name="x4", bufs=2) as x4_pool,
            tc.tile_pool(name="osb", bufs=2) as osb_pool,
            tc.tile_pool(name="ps", bufs=4, space="PSUM") as ps_pool,
        ):
            wsel_sb = singles.tile([128, NS // 2, 128], mmdt)
            nc.sync.dma_start(out=wsel_sb[:], in_=wsel_d[:, :, :])
            v_sb = singles.tile([NS, C], f32)
            nc.sync.dma_start(out=v_sb[:], in_=v_d[:, :])
            c0_sb = singles.tile([NS, 1], f32)
            nc.sync.dma_start(out=c0_sb[:], in_=c0_d[:, :])
            md_sb = singles.tile([NS, TW], f32)
            nc.sync.dma_start(out=md_sb[:], in_=md_d[:, :])

            for b in range(BPC):
                for q in range(NQ):
                    f0 = q * QF
                    base = x_d[b, :, f0 : f0 + 1, :]  # for offset only
                    # ---- 1. x_s [32, 3, 400]: partition=fsub, free=(c, fn)
                    xs = xs_pool.tile([NS, C, TW], f32)
                    src = bass.AP(
                        tensor=base.tensor,
                        offset=base.offset,
                        ap=[[TW, NS], [FN, C], [1, TW]],
                    )
                    nc.scalar.dma_start(out=xs[:], in_=src)
                    # ---- 2. softmax -> dd [32, 400]
                    t = sm_pool.tile([NS, TW], f32, tag="t")
                    nc.vector.tensor_scalar(
                        out=t[:],
                        in0=xs[:, 2, :],
                        scalar1=v_sb[:, 2:3],
                        scalar2=c0_sb[:, :],
                        op0=MULT,
                        op1=ADD,
                    )
                    for c in (1, 0):
                        nc.vector.scalar_tensor_tensor(
                            out=t[:],
                            in0=xs[:, c, :],
                            scalar=v_sb[:, c : c + 1],
                            in1=t[:],
                            op0=MULT,
                            op1=ADD,
                        )
                    e = sm_pool.tile([NS, TW], f32, tag="e")
                    nc.scalar.activation(out=e[:], in_=t[:], func=EXP)
                    ev = e[:].rearrange("p (a b) -> p a b", b=N)
                    z = sm_pool.tile([NS, FSUB], f32, tag="z")
                    nc.vector.reduce_sum(out=z[:], in_=ev, axis=AX)
                    r = sm_pool.tile([NS, FSUB], f32, tag="r")
                    nc.vector.reciprocal(out=r[:], in_=z[:])
                    em = sm_pool.tile([NS, TW], f32, tag="em")
                    nc.vector.tensor_tensor(
                        out=em[:], in0=e[:], in1=md_sb[:], op=MULT
                    )
                    dd = sm_pool.tile([NS, TW], f32, tag="dd")
                    rr = r[:, :]
                    r_bc = bass.AP(
                        tensor=rr.tensor,
                        offset=rr.offset,
                        ap=[rr.ap[0], [1, FSUB], [0, N]],
                    )
                    nc.vector.tensor_tensor(
                        out=dd[:], in0=em[:], in1=r_bc, op=MULT
                    )
                    # ---- 3. dmult [128, 400] = [dd; dd; dd; dd]
                    dmult = x4_pool.tile([128, TW], f32, tag="dmult")
                    for c in range(4):
                        eng = nc.sync if c % 2 == 0 else nc.scalar
                        eng.dma_start(
                            out=dmult[32 * c : 32 * (c + 1), :], in_=dd[:]
                        )
                    # ---- 4. x4 [128, 400]: rows 0:96 = x, rows 96:128 = 1.0
                    x4 = x4_pool.tile([128, TW], f32, tag="x4")
                    nc.vector.memset(x4[96:128, :], 1.0)
                    src4 = bass.AP(
                        tensor=base.tensor,
                        offset=base.offset,
                        ap=[[FN, C], [TW, NS], [1, TW]],
                    )
                    nc.sync.dma_start(out=x4[0:96, :], in_=src4)
                    # ---- 5. x4s = x4 * dmult
                    x4s = x4_pool.tile([128, TW], mmdt, tag="x4s")
                    nc.vector.tensor_tensor(
                        out=x4s[:], in0=x4[:], in1=dmult[:], op=MULT
                    )
                    # ---- 6./7. 16 matmuls + evictions
                    osb = osb_pool.tile([128, NS // 2, TW], f32)
                    for tp in range(NS // 2):
                        ph = ps_pool.tile([128, TW], f32, tag="ph")
                        nc.tensor.matmul(
                            ph[:, :],
                            wsel_sb[:, tp, :],
                            x4s[:, :],
                            start=True,
                            stop=True,
                        )
                        if tp % 2 == 0:
                            nc.vector.tensor_copy(osb[:, tp, :], ph[:, :])
                        else:
                            nc.scalar.copy(osb[:, tp, :], ph[:, :])
                    for jj in range(2):
                        osl = out_d[b, :, f0 + FSUB * jj : f0 + FSUB * jj + 1, :]
                        dst = bass.AP(
                            tensor=osl.tensor,
                            offset=osl.offset,
                            ap=[[FN, H], [2 * TW, NS // 2], [1, TW]],
                        )
                        nc.sync.dma_start(
                            out=dst, in_=osb[64 * jj : 64 * (jj + 1), :, :]
                        )
    nc.compile()
    return nc


def _get_nc():
    if "nc" not in _NC_CACHE:
        _NC_CACHE["nc"] = _build_nc()
    return _NC_CACHE["nc"]


def _make_in_maps(x, mask, W, bW, a1, a2, ab):
    x = np.ascontiguousarray(np.asarray(x, np.float32))
    mask = np.asarray(mask, np.float32)
    W = np.asarray(W, np.float32)
    bW = np.asarray(bW, np.float32)
    a2 = np.asarray(a2, np.float32)

    v = (W @ a2).astype(np.float32)                    # [C]
    c0 = np.float32(bW @ a2)
    md = np.diag(mask).astype(np.float32)              # [N]

    # wsel[row = 32c + fsub, tp, (jj, k)]:
    #   delta[fsub == 2 tp + jj] * (W[c, k] if c < 3 else bW[k])
    wsel = np.zeros((128, NS // 2, 128), np.float32)
    for tp in range(NS // 2):
        for jj in range(2):
            fsub = 2 * tp + jj
            for c in range(3):
                wsel[32 * c + fsub, tp, 64 * jj : 64 * jj + 64] = W[c]
            wsel[96 + fsub, tp, 64 * jj : 64 * jj + 64] = bW
    v_pp = np.tile(v[None, :], (NS, 1)).astype(np.float32)
    c0_pp = np.full((NS, 1), c0, np.float32)
    mdt = np.tile(md[None, :], (NS, FSUB)).astype(np.float32)

    in_maps = []
    for cix in range(NCORES):
        in_maps.append(
            {
                "x": np.ascontiguousarray(x[cix * BPC : (cix + 1) * BPC]),
                "wsel": wsel,
                "v_pp": v_pp,
                "c0_pp": c0_pp,
                "mdt": mdt,
            }
        )
    return in_maps


def run(x, mask, W, bW, a1, a2, ab, **run_kwargs):
    from concourse.bass_utils import run_bass_kernel_spmd

    nc = _get_nc()
    in_maps = _make_in_maps(x, mask, W, bW, a1, a2, ab)
    res = run_bass_kernel_spmd(nc, in_maps, core_ids=list(range(NCORES)), **run_kwargs)
    out = np.concatenate([res.results[i]["out"] for i in range(NCORES)], axis=0)
    return out, res


def kernel(x, mask, W, bW, a1, a2, ab):
    out, _ = run(x, mask, W, bW, a1, a2, ab)
    return out
